# revision 3
# baseline (speedup 1.0000x reference)
"""Trainium2 Bass kernel for quantized int8 per-channel Conv2d.

Reference semantics (fp32):
  x_f = (x_int8 - 7) * 0.01                      # per-tensor dequant
  w_f = (w_int8 - zp[cout]) * scale[cout]        # per-channel dequant
  y   = round(conv2d_valid(x_f, w_f) + bias[cout])  -> int32

Exact-integer factorization used here:
  conv(x_f, w_f) = 0.01*scale[o] * S(o, p),  S = conv((x-7), (w-zp[o]))
(x-7) in [-135,120] and (w-zp) in [-137,137] are exact in bf16; products
accumulate exactly in fp32 PSUM (|S| << 2^24 for this data).  The final
affine + round happens in fp32 with the 1.5*2^23 magic-number trick,
which rounds half-to-even exactly like jnp.round.

Sharding: data-parallel over batch N=32 across 8 cores (4 images each);
weights/scales/zeropoints/bias replicated.
"""

import numpy as np

import concourse.bass as bass
import concourse.mybir as mybir
from concourse import bacc
from concourse.tile import TileContext
from concourse.bass_utils import run_bass_kernel_spmd

# Problem shapes (hardcoded per contract)
N, CIN, H, W = 32, 256, 56, 56
COUT, KH, KW = 256, 3, 3
HO, WO = H - KH + 1, W - KW + 1          # 54, 54
NCORES = 8
NPER = N // NCORES                        # images per core
HW = H * W                                # 3136
XPAD = HW + 4                             # pad: tap (2,2) of last chunk reads 2 past
CHUNK = 9 * W                             # 504 positions = 9 output rows x 56 cols
NCHUNK = (HO * W) // CHUNK                # 6
KT = (CIN // 128)                         # 2 cin tiles
MT = COUT // 128                          # 2 cout tiles
TAPS = KH * KW                            # 9
MAGIC = 12582912.0                        # 1.5 * 2**23  (fp32 RNE rounding trick)
B_CHUNK = 3                               # chunks per matmul weight-reuse block

_CACHE = {}


def _build_program():
    nc = bacc.Bacc("TRN2", target_bir_lowering=False, debug=False,
                   num_devices=NCORES)
    dt = mybir.dt

    x_d = nc.dram_tensor("x", [NPER, CIN, H, W], dt.int8, kind="ExternalInput")
    wt_d = nc.dram_tensor("wt", [TAPS, CIN, COUT], dt.int8, kind="ExternalInput")
    sc_d = nc.dram_tensor("scales", [COUT], dt.float32, kind="ExternalInput")
    zp_d = nc.dram_tensor("zp", [COUT], dt.int32, kind="ExternalInput")
    bi_d = nc.dram_tensor("bias", [COUT], dt.float32, kind="ExternalInput")
    out_d = nc.dram_tensor("out", [NPER, COUT, HO, WO], dt.int32,
                           kind="ExternalOutput")

    with TileContext(nc) as tc:
        with (
            tc.tile_pool(name="const", bufs=1) as cpool,
            tc.tile_pool(name="xin", bufs=2) as xpool,
            tc.tile_pool(name="xbf", bufs=2) as xbpool,
            tc.tile_pool(name="psum", bufs=2 * B_CHUNK, space="PSUM") as ppool,
            tc.tile_pool(name="zpsum", bufs=1, space="PSUM") as zpool,
            tc.tile_pool(name="tmp", bufs=4) as tpool,
            tc.tile_pool(name="outb", bufs=3) as opool,
        ):
            # ---- one-time constants ----
            zp_i = cpool.tile([1, COUT], dt.int32)
            nc.sync.dma_start(out=zp_i[:, :], in_=zp_d[None, :])
            zp_f = cpool.tile([1, COUT], dt.float32)
            nc.vector.tensor_copy(zp_f[:, :], zp_i[:, :])

            ones = cpool.tile([1, 128], dt.float32)
            nc.vector.memset(ones[:, :], 1.0)
            zp_ps = zpool.tile([128, COUT], dt.float32)
            nc.tensor.matmul(zp_ps[:, :], ones[:, :], zp_f[:, :],
                             start=True, stop=True)
            zpb = cpool.tile([128, COUT], dt.float32)
            nc.vector.tensor_copy(zpb[:, :], zp_ps[:, :])

            # combined output scale 0.01*scale[o] and bias, one column per m-tile
            sc2 = cpool.tile([128, MT], dt.float32)
            nc.sync.dma_start(out=sc2[:, :], in_=sc_d.rearrange("(m p) -> p m", p=128))
            nc.scalar.mul(sc2[:, :], sc2[:, :], 0.01)
            bi2 = cpool.tile([128, MT], dt.float32)
            nc.sync.dma_start(out=bi2[:, :], in_=bi_d.rearrange("(m p) -> p m", p=128))

            m7 = cpool.tile([128, 1], dt.float32)
            nc.vector.memset(m7[:, :], -7.0)

            # ---- weights: int8 [tap, cin, cout] -> bf16 (w - zp) lhsT tiles ----
            wi8 = cpool.tile([128, TAPS * KT, COUT], dt.int8)
            for t in range(TAPS):
                for k in range(KT):
                    nc.sync.dma_start(
                        out=wi8[:, t * KT + k, :],
                        in_=wt_d[t, k * 128:(k + 1) * 128, :])
            wb = cpool.tile([128, TAPS * KT, COUT], dt.bfloat16)
            for t in range(TAPS):
                for k in range(KT):
                    nc.vector.tensor_tensor(
                        wb[:, t * KT + k, :], wi8[:, t * KT + k, :], zpb[:, :],
                        mybir.AluOpType.subtract)

            # ---- per-image pipeline ----
            for n in range(NPER):
                xi = xpool.tile([128, KT, XPAD], dt.int8)
                for k in range(KT):
                    nc.sync.dma_start(
                        out=xi[:, k, 0:HW],
                        in_=x_d[n, k * 128:(k + 1) * 128].rearrange("p h w -> p (h w)"))
                # x' = x - 7, exact in bf16 (pad cols hold finite garbage)
                xb = xbpool.tile([128, KT, XPAD], dt.bfloat16)
                nc.scalar.activation(xb[:, :, :], xi[:, :, :],
                                     mybir.ActivationFunctionType.Identity,
                                     bias=m7[:, :], scale=1.0)

                for m in range(MT):
                    ob = opool.tile([128, HO, WO], dt.int32)
                    for cb in range(NCHUNK // B_CHUNK):
                        ps = [ppool.tile([128, CHUNK], dt.float32,
                                         name="ps", tag="ps")
                              for _ in range(B_CHUNK)]
                        first = True
                        for k in range(KT):
                            for t in range(TAPS):
                                dh, dw = t // KW, t % KW
                                lhsT = wb[:, t * KT + k,
                                          m * 128:(m + 1) * 128]
                                for c0 in range(B_CHUNK):
                                    c = cb * B_CHUNK + c0
                                    off = c * CHUNK + dh * W + dw
                                    nc.tensor.matmul(
                                        ps[c0][:, :], lhsT,
                                        xb[:, k, off:off + CHUNK],
                                        start=first,
                                        stop=(k == KT - 1 and t == TAPS - 1))
                                first = False
                        for c0 in range(B_CHUNK):
                            c = cb * B_CHUNK + c0
                            # y = 0.01*scale*S + bias   (fp32, per-partition)
                            tmp = tpool.tile([128, CHUNK], dt.float32)
                            nc.scalar.activation(
                                tmp[:, :], ps[c0][:, :],
                                mybir.ActivationFunctionType.Identity,
                                bias=bi2[:, m:m + 1], scale=sc2[:, m:m + 1])
                            # round-to-nearest-even + extract valid 54 cols
                            t3 = tmp[:, :].rearrange("p (r w) -> p r w", w=W)
                            nc.vector.tensor_scalar(
                                ob[:, 9 * c:9 * (c + 1), :],
                                t3[:, :, 0:WO], MAGIC, MAGIC,
                                mybir.AluOpType.add, mybir.AluOpType.subtract)
                    nc.sync.dma_start(
                        out=out_d[n, m * 128:(m + 1) * 128], in_=ob[:, :, :])

    nc.compile()
    return nc


def kernel(**inputs) -> np.ndarray:
    x = np.ascontiguousarray(np.asarray(inputs["inputVec"], dtype=np.int8))
    w = np.asarray(inputs["weight"], dtype=np.int8)
    scales = np.ascontiguousarray(np.asarray(inputs["scales"], dtype=np.float32))
    zp = np.ascontiguousarray(np.asarray(inputs["zeropoints"], dtype=np.int32))
    bias = np.ascontiguousarray(np.asarray(inputs["bias"], dtype=np.float32))
    assert x.shape == (N, CIN, H, W) and w.shape == (COUT, CIN, KH, KW)

    # [cout, cin, kh, kw] -> [tap, cin, cout] so lhsT tiles DMA contiguously
    wt = np.ascontiguousarray(
        w.transpose(2, 3, 1, 0).reshape(TAPS, CIN, COUT))

    if "nc" not in _CACHE:
        _CACHE["nc"] = _build_program()
    nc = _CACHE["nc"]

    in_maps = [
        {"x": x[c * NPER:(c + 1) * NPER], "wt": wt, "scales": scales,
         "zp": zp, "bias": bias}
        for c in range(NCORES)
    ]
    res = run_bass_kernel_spmd(nc, in_maps, list(range(NCORES)))
    out = np.concatenate([res.results[c]["out"] for c in range(NCORES)], axis=0)
    return out
